# revision 42
# baseline (speedup 1.0000x reference)
"""BarrierNet Trainium2 kernel: 8-core data-parallel Bass/Tile implementation.

Takes full inputs, shards batch across 8 NeuronCores, returns full output.

Layout strategy (per core, S = 131072 samples):
  - obs loaded naturally: partition p of a span holds samples [base+64p, base+64p+64)
    (4KB contiguous per partition -> full DMA efficiency).
  - PE block-transposes [128,128] natural blocks into packed obsT (rows 16*t8+f).
  - The MLP matmul datapath runs in bfloat16 (weights pre-converted on host;
    activations written bf16 by the ScalarE silu, obsT converted inside the
    PSUM->SBUF copy). bf16 matmuls run at 1 cycle/row vs 4 for fp32 -> ~4x
    PE throughput; PSUM accumulation stays fp32, barrier math stays fp32.
  - MLP on PE in transposed activation layout:
      L1: K=32 matmuls with zero-padded w1 pairs (row strips, tile_position),
      L2: col-tiled K=128 matmuls (4 col strips of w2),
      L3: block-diagonal w3 -> u_nomT [8 rows = (2j+ch), 128].
  - silu via ScalarE Silu activation (PSUM->SBUF, bias = per-partition AP).
  - PE transpose-back of u_nomT -> natural u_nom planes.
  - Barrier math (dCVaR-CBF + closed-form QP) in natural layout on DVE:
    worst-case GMM mode is analytically the largest-sigma mode (means equal,
    sigma monotone in variance, CVaR coeff > 0), so only one mode is evaluated.
    sqrt via magic-seed + Newton rsqrt, projection division via DVE reciprocal.
  - Output assembled run-major: partition p holds samples 64p..64p+63 interleaved
    (x,y) -> 512-byte contiguous runs per partition -> efficient store.
"""
import sys

sys.path.insert(0, '/opt/trn_rl_repo')

from collections import deque
from contextlib import ExitStack

import numpy as np

import concourse.bass as bass  # noqa: F401
import concourse.tile as tile
from concourse import bacc, mybir
from concourse.bass_utils import run_bass_kernel_spmd
from concourse.masks import make_identity

N_CORES = 8
B = 1_048_576
NF, H1, H2, NC = 16, 128, 32, 2
S = B // N_CORES              # samples per core
SAFE_DIST = 0.8
ALPHA = 2.0
CVAR_COEFF = 1.7549833193248685
SIG_MAX_VAR = 0.3 * 0.3       # largest GMM mode variance (worst-case mode)
EPS_SIG = 1e-8
EPS_DIV = 1e-12

# Schraudolph exp: e^x ~= bitcast_f32(int32(A_EXP*x + B_EXP))
A_EXP = 12102203.161561485    # 2^23 / ln 2
B_EXP = 1064993000.0          # offset tuned for min |silu| error (~6.7e-3)

TR = 64                        # samples per partition run
V = 128 * TR                   # natural span = 8192 samples
FP32 = mybir.dt.float32
BF16 = mybir.dt.bfloat16
I32 = mybir.dt.int32

OFF_T8 = -1                    # t8 group computed on DVE+Pool instead of Act
C_DVE = 88                     # barrier column split: DVE gets [0:C), Pool rest
SPAN_GRP = 2

_cached = {}


def build(s_samples=S, n_devices=N_CORES):
    nc = bacc.Bacc("TRN2", target_bir_lowering=False, debug=False,
                   num_devices=n_devices)
    obs_ap = nc.dram_tensor("obs", [s_samples, NF], FP32, kind="ExternalInput").ap()
    w1p0_ap = nc.dram_tensor("w1pad0", [128, 128], BF16, kind="ExternalInput").ap()
    w1p1_ap = nc.dram_tensor("w1pad1", [128, 128], BF16, kind="ExternalInput").ap()
    w2r_ap = nc.dram_tensor("w2rep", [128, 128], BF16, kind="ExternalInput").ap()
    w3b_ap = nc.dram_tensor("w3rep", [128, 2], BF16, kind="ExternalInput").ap()
    b1_ap = nc.dram_tensor("b1c", [128, 1], FP32, kind="ExternalInput").ap()
    b2_ap = nc.dram_tensor("b2rep", [128, 1], FP32, kind="ExternalInput").ap()
    b3_ap = nc.dram_tensor("b3bc", [128, 2], FP32, kind="ExternalInput").ap()
    bv_ap = nc.dram_tensor("bvec1", [128, 1], FP32, kind="ExternalInput").ap()
    out_ap = nc.dram_tensor("out", [s_samples, NC], FP32, kind="ExternalOutput").ap()

    with tile.TileContext(nc) as tc, ExitStack() as ctx:
        Kernel(ctx, tc, out_ap, obs_ap, (w1p0_ap, w1p1_ap), w2r_ap, w3b_ap,
               b1_ap, b2_ap, b3_ap, bv_ap, s_samples).emit()
    nc.compile()
    return nc


class Kernel:
    def __init__(self, ctx, tc, out_ap, obs_ap, w1p_aps, w2r_ap, w3b_ap,
                 b1_ap, b2_ap, b3_ap, bv_ap, s_samples):
        self.tc = tc
        self.nc = tc.nc
        self.out_ap = out_ap
        self.obs_ap = obs_ap
        self.s_samples = s_samples
        self.nspan = s_samples // V
        nc = self.nc

        self.const = ctx.enter_context(tc.tile_pool(name="const", bufs=1))
        self.nat_pool = ctx.enter_context(tc.tile_pool(name="nat", bufs=3))
        self.obsT_pool = ctx.enter_context(tc.tile_pool(name="obsT", bufs=2))
        self.y1_pool = ctx.enter_context(tc.tile_pool(name="y1", bufs=2))
        self.y2_pool = ctx.enter_context(tc.tile_pool(name="y2", bufs=2))
        self.scr_pool = ctx.enter_context(tc.tile_pool(name="scr", bufs=2))
        self.plane_pool = ctx.enter_context(tc.tile_pool(name="plane", bufs=2))
        self.outb_pool = ctx.enter_context(tc.tile_pool(name="outb", bufs=2))

        # PSUM: y1 2x2 banks + y2 2 + tp 1 + un 1 = 8 banks exactly.
        self.ps_y1 = ctx.enter_context(
            tc.tile_pool(name="ps_y1", bufs=2, space="PSUM"))
        self.ps_y2 = ctx.enter_context(
            tc.tile_pool(name="ps_y2", bufs=1, space="PSUM"))
        self.ps_tr = ctx.enter_context(
            tc.tile_pool(name="ps_tr", bufs=1, space="PSUM"))
        self.ps_un = ctx.enter_context(
            tc.tile_pool(name="ps_un", bufs=1, space="PSUM"))

        # constants
        c = self.const
        self.w1p0 = c.tile([128, 128], BF16)
        self.w1p1 = c.tile([128, 128], BF16)
        self.w2rep = c.tile([128, 128], BF16)
        self.w3rep = c.tile([128, 2], BF16)
        self.b1c = c.tile([128, 1], FP32)
        self.b2rep = c.tile([128, 1], FP32)
        self.b3bc = c.tile([128, 2], FP32)
        self.bvec1 = c.tile([128, 1], FP32)
        self.ident = c.tile([128, 128], FP32)
        self._const_dmas = [
            (self.w1p0, w1p_aps[0]), (self.w1p1, w1p_aps[1]),
            (self.w2rep, w2r_ap), (self.w3rep, w3b_ap),
            (self.b1c, b1_ap), (self.b2rep, b2_ap),
            (self.b3bc, b3_ap), (self.bvec1, bv_ap)]
        self.w1pads = (self.w1p0, self.w1p1)

        self.spans = {}     # span -> dict of live SBUF tiles
        self.groups = {}    # sg -> dict of plane tiles

    # ---------------- per-span front: load, extract, transpose ----------
    def front_dma(self, k):
        nc = self.nc
        sg, sl = divmod(k, SPAN_GRP)
        if sl == 0:
            PW = SPAN_GRP * TR
            g = {}
            for t in ("relx", "rely", "hvx", "hvy", "unx", "uny"):
                g[t] = self.plane_pool.tile([128, PW], FP32, tag=t, name=t)
            self.groups[sg] = g
        g = self.groups[sg]

        base = k * V
        obs_nat = self.nat_pool.tile([128, TR * NF], FP32, tag="obs_nat")
        src = self.obs_ap[base:base + V, :].rearrange("(p t) f -> p (t f)", p=128)
        nc.sync.dma_start(obs_nat[:], src)
        self.spans[k] = dict(obs_nat=obs_nat)

    def extract(self, k):
        # barrier input extraction (GpSimd; natural planes). Emitted late in
        # the previous span so it never precedes the offload-tail Pool ops.
        G_ = self.nc.gpsimd
        sg, sl = divmod(k, SPAN_GRP)
        g = self.groups[sg]
        obs_nat = self.spans[k]["obs_nat"]
        ob3 = obs_nat[:].rearrange("p (t f) -> p t f", f=NF)
        pl_sl = slice(sl * TR, (sl + 1) * TR)
        G_.tensor_copy(g["relx"][:, pl_sl], ob3[:, :, 6])
        G_.tensor_copy(g["rely"][:, pl_sl], ob3[:, :, 7])
        G_.tensor_copy(g["hvx"][:, pl_sl], ob3[:, :, 8])
        G_.tensor_copy(g["hvy"][:, pl_sl], ob3[:, :, 9])

    def front_T(self, k):
        nc, V_ = self.nc, self.nc.vector
        sp = self.spans[k]
        obs_nat = sp["obs_nat"]
        # PE transpose natural -> packed obsT (rows 16*t8+f)
        obsT = self.obsT_pool.tile([128, 1024], BF16, tag="obsT")
        for half in range(2):
            tp = self.ps_tr.tile([128, 512], FP32, tag="tp")
            for ci in range(4):
                col = half * 4 + ci
                nc.tensor.transpose(
                    tp[:, ci * 128:(ci + 1) * 128],
                    obs_nat[:, col * 128:(col + 1) * 128],
                    self.ident[:])
            V_.tensor_copy(obsT[:, half * 512:(half + 1) * 512], tp[:])

        # layouts: y1sT col = (t8*2+h)*512 + n ; y2sT col = (h*2+sub)*512 + n
        sp["obsT"] = obsT
        sp["y1sT"] = self.y1_pool.tile([128, 8192], BF16, tag="y1sT",
                                       name="y1sT")
        sp["y2sT"] = self.y2_pool.tile([128, 2048], BF16, tag="y2sT",
                                       name="y2sT")

    # ---------------- L1 tile: one t8 group, both halves + silu --------
    def l1_tile(self, k, t8):
        nc, V_, G_ = self.nc, self.nc.vector, self.nc.gpsimd
        ALU = mybir.AluOpType
        SILU = mybir.ActivationFunctionType.Silu
        sp = self.spans[k]
        obsT, y1sT = sp["obsT"], sp["y1sT"]
        y1v = y1sT[:].rearrange("q (t8 hn) -> q t8 hn", t8=8)

        y1q = self.ps_y1.tile([128, 1024], FP32, tag="y1q")
        par, s4 = t8 % 2, t8 // 2
        for h in range(2):
            nc.tensor.matmul(
                y1q[:, h * 512:(h + 1) * 512],
                self.w1pads[par][32 * s4:32 * s4 + 32, :],
                obsT[32 * s4:32 * s4 + 32,
                     h * 512:(h + 1) * 512],
                start=True, stop=True,
                tile_position=(32 * s4, 0))
        if t8 != OFF_T8:
            nc.scalar.activation(y1v[:, t8], y1q[:], SILU,
                                 bias=self.b1c[:, 0:1], scale=1.0)
        else:
            # Whole t8 group offloaded: h0 -> all-DVE Schraudolph silu chain
            # (reciprocal_approx_fast), h1 -> GpSimd chain closed by a float
            # divide. The four PSUM reads (bits & z per half) are emitted
            # inline so the y1q bank releases early; the SBUF-only tails are
            # emitted later (offload_tail), off the critical path.
            tiles = {}
            for h in range(2):
                zt = self.scr_pool.tile([128, 512], FP32, tag="zt%d" % h,
                                        name="zt")
                dt_ = self.scr_pool.tile([128, 512], FP32, tag="dt%d" % h,
                                         name="dt")
                cs = slice(h * 512, (h + 1) * 512)
                V_.tensor_scalar(dt_[:], y1q[:, cs], -A_EXP,
                                 self.bvec1[:, 0:1], ALU.mult, ALU.add)
                V_.tensor_scalar(zt[:], y1q[:, cs], self.b1c[:, 0:1],
                                 None, ALU.add)
                tiles[h] = (zt, dt_)
            self.spans[k]["off"] = (tiles, y1sT)

    def offload_tail(self, k):
        V_, G_ = self.nc.vector, self.nc.gpsimd
        ALU = mybir.AluOpType
        tiles, y1sT = self.spans[k].pop("off")
        # h0 on DVE
        zt, dt_ = tiles[0]
        it_ = self.scr_pool.tile([128, 512], FP32, tag="it0", name="it")
        V_.tensor_copy(it_[:].bitcast(I32), dt_[:])          # round -> int
        V_.tensor_scalar(dt_[:], it_[:], 1.0, None, ALU.add)  # d = 1 + u
        V_.reciprocal_approx_fast(dt_[:], dt_[:])
        V_.tensor_mul(y1sT[:, (2 * OFF_T8) * 512:(2 * OFF_T8 + 1) * 512],
                      zt[:], dt_[:])
        # h1 on GpSimd (float divide)
        zt, dt_ = tiles[1]
        it_ = self.scr_pool.tile([128, 512], FP32, tag="it1", name="it")
        G_.tensor_copy(it_[:].bitcast(I32), dt_[:])
        G_.tensor_scalar(dt_[:], it_[:], 1.0, None, ALU.add)
        G_.tensor_tensor(y1sT[:, (2 * OFF_T8 + 1) * 512:(2 * OFF_T8 + 2) * 512],
                         zt[:], dt_[:], ALU.divide)

    # ---------------- L2 / L3 emission helpers --------------------------
    def l2_mm(self, k, h, sub):
        nc = self.nc
        sp = self.spans[k]
        y1sT = sp["y1sT"]
        if sub == 0:
            sp["y2q_%d" % h] = self.ps_y2.tile([128, 1024], FP32, tag="y2q",
                                               name="y2q")
        y2q = sp["y2q_%d" % h]
        for j in range(4):
            t8 = 4 * sub + j
            nc.tensor.matmul(
                y2q[32 * j:32 * j + 32, sub * 512:(sub + 1) * 512],
                self.w2rep[:, 32 * j:32 * j + 32],
                y1sT[:, (t8 * 2 + h) * 512:
                     (t8 * 2 + h + 1) * 512],
                start=True, stop=True,
                tile_position=(0, 32 * j))

    def l2_act(self, k, h):
        SILU = mybir.ActivationFunctionType.Silu
        sp = self.spans[k]
        self.nc.scalar.activation(
            sp["y2sT"][:, h * 1024:(h + 1) * 1024], sp["y2q_%d" % h][:],
            SILU, bias=self.b2rep[:, 0:1], scale=1.0)

    def l3_mm(self, k, h, sub):
        # L3 with y2 blocks as the STATIONARY operand: out partitions become
        # the natural sample partitions -- no transpose-back, no PSUM->SBUF
        # copy. out[p, ch] for sample t = 32h + 8*ci + 4*sub + j lands at
        # unq[:, 2t:2t+2].
        nc = self.nc
        sp = self.spans[k]
        y2sT = sp["y2sT"]
        if h == 0 and sub == 0:
            sp["unq"] = self.ps_un.tile([128, 128], FP32, tag="un",
                                        name="unq")
        unq = sp["unq"]
        for ci in range(4):
            base_col = (h * 2 + sub) * 512 + ci * 128
            for j in range(4):
                t = 32 * h + 8 * ci + 4 * sub + j
                nc.tensor.matmul(
                    unq[:, 2 * t:2 * t + 2],
                    y2sT[32 * j:32 * j + 32,
                         base_col:base_col + 128],
                    self.w3rep[32 * j:32 * j + 32, :],
                    start=True, stop=True,
                    tile_position=(32 * j, 0))

    def planes(self, k):
        V_ = self.nc.vector
        ALU = mybir.AluOpType
        sp = self.spans[k]
        sg, sl = divmod(k, SPAN_GRP)
        g = self.groups[sg]
        pl_sl = slice(sl * TR, (sl + 1) * TR)
        uv = sp["unq"][:].rearrange("p (t ch) -> p t ch", ch=2)
        V_.tensor_scalar(g["unx"][:, pl_sl], uv[:, :, 0],
                         self.b3bc[:, 0:1], None, ALU.add)
        V_.tensor_scalar(g["uny"][:, pl_sl], uv[:, :, 1],
                         self.b3bc[:, 1:2], None, ALU.add)
        self.spans.pop(k, None)

    # ---------------- barrier math + store (per span-group) -------------
    def enqueue_barrier(self, sg):
        """Enqueue the barrier as one closure per elementwise op so it drains
        interleaved with per-span work instead of bursting on DVE/GpSimd."""
        nc, V_, G_ = self.nc, self.nc.vector, self.nc.gpsimd
        ALU = mybir.AluOpType
        PW = SPAN_GRP * TR
        g = self.groups[sg]

        def alloc():
            g["outb"] = self.outb_pool.tile([128, 2 * PW], FP32, tag="outb",
                                            name="outb")
            tmp = self.plane_pool
            for t in ("sx", "sy", "rnsq", "rdm2", "sig", "q1", "viol",
                      "gnsq", "coef"):
                g[t] = tmp.tile([128, PW], FP32, tag=t, name=t)

        self.qs.append(alloc)

        def op(fn):
            self.qs.append(fn)

        relx = lambda cs: g["relx"][:, cs]
        rely = lambda cs: g["rely"][:, cs]
        hvx = lambda cs: g["hvx"][:, cs]
        hvy = lambda cs: g["hvy"][:, cs]
        unx = lambda cs: g["unx"][:, cs]
        uny = lambda cs: g["uny"][:, cs]
        sx = lambda cs: g["sx"][:, cs]
        sy = lambda cs: g["sy"][:, cs]
        rnsq = lambda cs: g["rnsq"][:, cs]
        rdm2 = lambda cs: g["rdm2"][:, cs]
        sig = lambda cs: g["sig"][:, cs]
        q1 = lambda cs: g["q1"][:, cs]
        viol = lambda cs: g["viol"][:, cs]
        gnsq = lambda cs: g["gnsq"][:, cs]
        coef = lambda cs: g["coef"][:, cs]

        def stage1(EN, cs):
            op(lambda: EN.tensor_mul(sx(cs), relx(cs), relx(cs)))
            op(lambda: EN.tensor_mul(sy(cs), rely(cs), rely(cs)))
            op(lambda: EN.tensor_add(rnsq(cs), sx(cs), sy(cs)))
            op(lambda: EN.tensor_mul(sx(cs), hvx(cs), relx(cs)))
            op(lambda: EN.tensor_mul(sy(cs), hvy(cs), rely(cs)))
            op(lambda: EN.tensor_add(rdm2(cs), sx(cs), sy(cs)))
            # sigma = sqrt(x), x = 4*var*rnsq + eps_sig; magic rsqrt + 2 NR
            op(lambda: EN.tensor_scalar(sig(cs), rnsq(cs), 4.0 * SIG_MAX_VAR,
                                        EPS_SIG, ALU.mult, ALU.add))
            op(lambda: EN.tensor_copy(sx(cs), sig(cs).bitcast(I32)))
            op(lambda: EN.tensor_scalar(sx(cs), sx(cs), -0.5, 1597463007.0,
                                        ALU.mult, ALU.add))
            op(lambda: EN.tensor_copy(coef(cs).bitcast(I32), sx(cs)))
            for _ in range(1):
                op(lambda: EN.tensor_mul(sx(cs), coef(cs), coef(cs)))
                op(lambda: EN.tensor_mul(sx(cs), sx(cs), sig(cs)))
                op(lambda: EN.tensor_scalar(sx(cs), sx(cs), -0.5, 1.5,
                                            ALU.mult, ALU.add))
                op(lambda: EN.tensor_mul(coef(cs), coef(cs), sx(cs)))
            op(lambda: EN.tensor_mul(sig(cs), sig(cs), coef(cs)))
            op(lambda: EN.tensor_scalar(sig(cs), sig(cs), CVAR_COEFF,
                                        2.0 * SAFE_DIST ** 2,
                                        ALU.mult, ALU.add))
            # q1 = rdm2 - rnsq - dot(rel, u_nom)
            op(lambda: EN.tensor_sub(q1(cs), rdm2(cs), rnsq(cs)))
            op(lambda: EN.tensor_mul(sx(cs), relx(cs), unx(cs)))
            op(lambda: EN.tensor_mul(sy(cs), rely(cs), uny(cs)))
            op(lambda: EN.tensor_add(sx(cs), sx(cs), sy(cs)))
            op(lambda: EN.tensor_sub(q1(cs), q1(cs), sx(cs)))
            # viol = 2*q1 + sig ; gnsq = 4*rnsq + eps
            op(lambda: EN.tensor_scalar(q1(cs), q1(cs), 2.0, None, ALU.mult))
            op(lambda: EN.tensor_add(viol(cs), q1(cs), sig(cs)))
            op(lambda: EN.tensor_scalar(gnsq(cs), rnsq(cs), 4.0, EPS_DIV,
                                        ALU.mult, ALU.add))
            op(lambda: EN.tensor_scalar(viol(cs), viol(cs), 0.0, 2.0,
                                        ALU.max, ALU.mult))

        def stage3(EN, cs):
            op(lambda: EN.tensor_mul(coef(cs), viol(cs), gnsq(cs)))
            op(lambda: EN.tensor_mul(sx(cs), coef(cs), relx(cs)))
            op(lambda: EN.tensor_mul(sy(cs), coef(cs), rely(cs)))

            def ox(cs, ch):
                return g["outb"][:].rearrange(
                    "p (w ch) -> p w ch", ch=2)[:, cs, ch]

            op(lambda: EN.tensor_add(ox(cs, 0), unx(cs), sx(cs)))
            op(lambda: EN.tensor_add(ox(cs, 1), uny(cs), sy(cs)))

        dve_cs = slice(0, C_DVE)
        gp_cs = slice(C_DVE, PW)
        stage1(G_, gp_cs)
        stage1(V_, dve_cs)
        op(lambda: V_.reciprocal(g["gnsq"][:], g["gnsq"][:]))
        stage3(V_, dve_cs)
        stage3(G_, gp_cs)

        def store(sl):
            base = (sg * SPAN_GRP + sl) * V
            dst = self.out_ap[base:base + V, :].rearrange(
                "(p t) c -> p (t c)", p=128)
            nc.sync.dma_start(dst, g["outb"][:, sl * 2 * TR:(sl + 1) * 2 * TR])

        for sl in range(SPAN_GRP):
            op(lambda sl=sl: store(sl))
        op(lambda: self.groups.pop(sg))

    # ---------------- pipelined emission --------------------------------
    def drain_qs(self, ns):
        for _ in range(ns):
            if not self.qs:
                return
            self.qs.popleft()()

    def emit(self):
        self.qb = deque()
        self.qs = deque()
        self.stores = []
        self.inloop_done = 0
        n = self.nspan
        self.front_dma(0)
        for t, ap in self._const_dmas:
            self.nc.sync.dma_start(t[:], ap[:])
        make_identity(self.nc, self.ident[:])
        self.front_T(0)
        for k in range(n):
            for t8 in range(8):
                self.l1_tile(k, t8)
                if t8 == 0 and k > 0 and OFF_T8 >= 0:
                    self.offload_tail(k - 1)
                elif t8 == 2 and k + 1 < n:
                    self.front_dma(k + 1)
                elif t8 == 3 and k > 0:
                    self.l2_mm(k - 1, 0, 1)
                    self.l2_act(k - 1, 0)
                elif t8 == 4 and k > 0:
                    self.l2_mm(k - 1, 1, 0)
                elif t8 == 5:
                    if k + 1 < n:
                        self.front_T(k + 1)
                    if k > 0:
                        self.l2_mm(k - 1, 1, 1)
                        self.l2_act(k - 1, 1)
                elif t8 == 6 and k > 0:
                    self.l3_mm(k - 1, 0, 0)
                    self.l3_mm(k - 1, 0, 1)
                elif t8 == 7:
                    self.l2_mm(k, 0, 0)
                    if k > 0:
                        self.l3_mm(k - 1, 1, 0)
                        self.l3_mm(k - 1, 1, 1)
                        self.planes(k - 1)
                    self.extract(k)
                if t8 == 7:
                    import os
                    budget = int(os.environ.get("INLOOP_QS", "0"))
                    take = max(0, min(21, budget - self.inloop_done))
                    self.inloop_done += min(take, len(self.qs))
                    self.drain_qs(take)
            if k >= 2 and (k - 2) % SPAN_GRP == 0:
                self.enqueue_barrier((k - 2) // SPAN_GRP)
        # tail: span n-1's back work, remaining barriers, flush
        if OFF_T8 >= 0:
            self.offload_tail(n - 1)
        self.l2_mm(n - 1, 0, 1)
        self.l2_act(n - 1, 0)
        self.l2_mm(n - 1, 1, 0)
        self.l2_mm(n - 1, 1, 1)
        self.l2_act(n - 1, 1)
        for h in range(2):
            for sub in range(2):
                self.l3_mm(n - 1, h, sub)
        self.planes(n - 1)
        done = (n - 2 - SPAN_GRP) // SPAN_GRP + 1 if n > 2 else 0
        for sg in range(done, n // SPAN_GRP):
            self.enqueue_barrier(sg)
        while self.qs:
            self.drain_qs(16)


def prep_consts(w1, b1, w2, b2, w3, b3):
    import ml_dtypes
    w1pad0 = np.zeros((128, 128), np.float32)
    w1pad1 = np.zeros((128, 128), np.float32)
    w2rep = np.zeros((128, 128), np.float32)
    for s4 in range(4):
        w1pad0[32 * s4:32 * s4 + 16, :] = w1.T          # even t8 groups
        w1pad1[32 * s4 + 16:32 * s4 + 32, :] = w1.T     # odd t8 groups
    for j in range(4):
        w2rep[:, 32 * j:32 * j + 32] = w2.T
    w3rep = np.tile(np.asarray(w3, np.float32).T, (4, 1))   # [128, 2]
    b1cv = np.asarray(b1, np.float32).reshape(128, 1)
    bf = ml_dtypes.bfloat16
    return dict(
        w1pad0=w1pad0.astype(bf), w1pad1=w1pad1.astype(bf),
        w2rep=w2rep.astype(bf), w3rep=w3rep.astype(bf),
        b1c=b1cv,
        b2rep=np.tile(np.asarray(b2, np.float32), 4).reshape(128, 1),
        b3bc=np.tile(np.asarray(b3, np.float32).reshape(1, 2), (128, 1)),
        bvec1=(B_EXP - A_EXP * b1cv).astype(np.float32))


def kernel(obs, w1, b1, w2, b2, w3, b3):
    obs = np.asarray(obs, np.float32)
    consts = prep_consts(np.asarray(w1, np.float32), np.asarray(b1, np.float32),
                         np.asarray(w2, np.float32), np.asarray(b2, np.float32),
                         np.asarray(w3, np.float32), np.asarray(b3, np.float32))
    if "nc" not in _cached:
        _cached["nc"] = build()
    nc = _cached["nc"]
    in_maps = []
    for k in range(N_CORES):
        m = {"obs": np.ascontiguousarray(obs[k * S:(k + 1) * S])}
        m.update(consts)
        in_maps.append(m)
    res = run_bass_kernel_spmd(nc, in_maps, list(range(N_CORES)))
    out = np.empty((B, NC), np.float32)
    for k in range(N_CORES):
        out[k * S:(k + 1) * S] = res.results[k]["out"]
    return out
